# revision 16
# baseline (speedup 1.0000x reference)
"""Trainium2 Bass kernel for nn_DecoderLSTM (topk_masking).

Sequential 10-step LSTM decoder with attention logits over a large encoder
tensor. Data-parallel over batch across 8 NeuronCores (16 rows/core);
weights replicated. Per step each core re-streams its 128 MiB encoder shard
from HBM (memory-bound regime) and computes, per batch row b:

    gates = x @ W_ih.T + b_ih + h @ W_hh.T + b_hh       (PE, transposed layout)
    h, c  = LSTM cell update                             (ACT sigmoid/tanh + DVE)
    q     = h @ Wq.T + bq                                (PE)
    logits[b, n] = <enc[b, n, :], q[b]>                  (DVE tensor_tensor_reduce)
    top3  = top-3 indices of logits[b]                   (DVE max8/max_index)
    x'    = enc[b, top3, :] flattened                    (GPSIMD indirect DMA)

Layout notes:
  - enc streamed in natural layout tiles [128 n, 256 d]; fused multiply+reduce
    (tensor_tensor_reduce) gives one logits chunk [128, 1] per tile.
  - per-b logits columns [128, 64] are PE-transposed (2 b per 128x128 block)
    so rows of 128 consecutive n become contiguous, then DMA'd both to the
    DRAM output and into an SBUF [16, 8192] tile for the top-k.
  - small LSTM/query matmuls run in transposed layout [features, b] so PE can
    consume them without runtime transposes of activations.
"""

import sys

sys.path.insert(0, "/opt/trn_rl_repo")

import numpy as np

B = 128
N = 8192
HD = 256
S = 10
NCORES = 8
B_LOC = B // NCORES  # 16


def _build_nc(n=N, s=S, cpd=16, enc_bufs=7):
    """Build the per-core Bass program. n = encoder length, s = steps,
    cpd = 128-row chunks per DMA (cpd*128 rows per dma_start)."""
    import concourse.bacc as bacc
    import concourse.mybir as mybir
    import concourse.tile as tile
    from concourse import bass

    dt = mybir.dt
    f32 = dt.float32
    u32 = dt.uint32
    Alu = mybir.AluOpType
    Act = mybir.ActivationFunctionType

    nchunk = n // 128          # logits chunks per b
    ndma = nchunk // cpd       # dma_starts per b
    assert nchunk % cpd == 0 and nchunk % 2 == 0

    nc = bacc.Bacc(None, target_bir_lowering=False, debug=False)

    enc = nc.declare_dram_parameter("enc", [B_LOC, n, HD], f32, isOutput=False)
    w_ihT = nc.declare_dram_parameter("w_ihT", [6, 128, 4 * HD], f32, isOutput=False)
    w_hhT = nc.declare_dram_parameter("w_hhT", [2, 128, 4 * HD], f32, isOutput=False)
    wqT = nc.declare_dram_parameter("wqT", [2, 128, HD], f32, isOutput=False)
    bias_g = nc.declare_dram_parameter("bias_g", [128, 8], f32, isOutput=False)
    bias_q = nc.declare_dram_parameter("bias_q", [128, 2], f32, isOutput=False)
    x0T = nc.declare_dram_parameter("x0T", [6, 128, B_LOC], f32, isOutput=False)
    h0T = nc.declare_dram_parameter("h0T", [2, 128, B_LOC], f32, isOutput=False)
    c0T = nc.declare_dram_parameter("c0T", [2, 128, B_LOC], f32, isOutput=False)
    ident = nc.declare_dram_parameter("ident", [128, 128], f32, isOutput=False)
    bsel = nc.declare_dram_parameter("bsel", [B_LOC, B_LOC, 128], f32, isOutput=False)
    row_base = nc.declare_dram_parameter("row_base", [B_LOC, 1], u32, isOutput=False)

    logits_out = nc.declare_dram_parameter(
        "logits_out", [s, B_LOC, n], f32, isOutput=True
    )
    idx_out = nc.declare_dram_parameter("idx_out", [s, B_LOC, 3], u32, isOutput=True)

    with tile.TileContext(nc) as tc:
        with (
            tc.tile_pool(name="const", bufs=1) as constp,
            tc.tile_pool(name="state", bufs=1) as statep,
            tc.tile_pool(name="enc", bufs=enc_bufs) as encp,
            tc.tile_pool(name="work", bufs=2) as workp,
            tc.tile_pool(name="small", bufs=2) as smallp,
            tc.tile_pool(name="ps_mm", bufs=1, space="PSUM") as ps_mm,
            tc.tile_pool(name="ps_tp", bufs=2, space="PSUM") as ps_tp,
        ):
            # ---- constants / weights resident in SBUF ----
            w_ihT_t = constp.tile([128, 6, 4 * HD], f32)
            w_hhT_t = constp.tile([128, 2, 4 * HD], f32)
            wqT_t = constp.tile([128, 2, HD], f32)
            bias_g_t = constp.tile([128, 8], f32)
            bias_q_t = constp.tile([128, 2], f32)
            ident_t = constp.tile([128, 128], f32)
            bsel_t = constp.tile([B_LOC, B_LOC, 128], f32)
            row_base_t = constp.tile([B_LOC, 1], u32)
            for kc in range(6):
                nc.sync.dma_start(w_ihT_t[:, kc, :], w_ihT[kc, :, :])
            for kc in range(2):
                nc.sync.dma_start(w_hhT_t[:, kc, :], w_hhT[kc, :, :])
                nc.sync.dma_start(wqT_t[:, kc, :], wqT[kc, :, :])
            nc.sync.dma_start(bias_g_t[:], bias_g[:])
            nc.sync.dma_start(bias_q_t[:], bias_q[:])
            nc.sync.dma_start(ident_t[:], ident[:])
            nc.sync.dma_start(bsel_t[:], bsel[:])
            nc.sync.dma_start(row_base_t[:], row_base[:])

            # ---- persistent state ----
            xT = statep.tile([128, 6, B_LOC], f32)    # input (transposed chunks)
            hT = statep.tile([128, 2, B_LOC], f32)
            cT = statep.tile([128, 2, B_LOC], f32)
            logits16 = statep.tile([B_LOC, n], f32)   # assembled logits for top-k
            qnat = statep.tile([B_LOC, HD], f32)      # queries, natural layout
            for kc in range(6):
                nc.sync.dma_start(xT[:, kc, :], x0T[kc, :, :])
            for kc in range(2):
                nc.sync.dma_start(hT[:, kc, :], h0T[kc, :, :])
                nc.sync.dma_start(cT[:, kc, :], c0T[kc, :, :])

            for t in range(s):
                # ---------- LSTM cell (transposed: [feature, b]) ----------
                psum_g = ps_mm.tile([128, 8, B_LOC], f32, space="PSUM", tag="gates")
                for m in range(8):
                    msl = slice(m * 128, (m + 1) * 128)
                    for kc in range(6):
                        nc.tensor.matmul(
                            psum_g[:, m, :],
                            lhsT=w_ihT_t[:, kc, msl],
                            rhs=xT[:, kc, :],
                            start=(kc == 0),
                            stop=False,
                        )
                    for kc in range(2):
                        nc.tensor.matmul(
                            psum_g[:, m, :],
                            lhsT=w_hhT_t[:, kc, msl],
                            rhs=hT[:, kc, :],
                            start=False,
                            stop=(kc == 1),
                        )
                sI = smallp.tile([128, 2, B_LOC], f32, tag="sI")
                sF = smallp.tile([128, 2, B_LOC], f32, tag="sF")
                tG = smallp.tile([128, 2, B_LOC], f32, tag="tG")
                sO = smallp.tile([128, 2, B_LOC], f32, tag="sO")
                for m in range(2):
                    nc.scalar.activation(
                        sI[:, m, :], psum_g[:, m, :], Act.Sigmoid,
                        bias=bias_g_t[:, m : m + 1],
                    )
                    nc.scalar.activation(
                        sF[:, m, :], psum_g[:, 2 + m, :], Act.Sigmoid,
                        bias=bias_g_t[:, 2 + m : 3 + m],
                    )
                    nc.scalar.activation(
                        tG[:, m, :], psum_g[:, 4 + m, :], Act.Tanh,
                        bias=bias_g_t[:, 4 + m : 5 + m],
                    )
                    nc.scalar.activation(
                        sO[:, m, :], psum_g[:, 6 + m, :], Act.Sigmoid,
                        bias=bias_g_t[:, 6 + m : 7 + m],
                    )
                fc = smallp.tile([128, 2, B_LOC], f32, tag="fc")
                ig = smallp.tile([128, 2, B_LOC], f32, tag="ig")
                nc.vector.tensor_mul(fc[:], sF[:], cT[:])
                nc.vector.tensor_mul(ig[:], sI[:], tG[:])
                nc.vector.tensor_add(cT[:], fc[:], ig[:])
                thc = smallp.tile([128, 2, B_LOC], f32, tag="thc")
                nc.scalar.activation(thc[:], cT[:], Act.Tanh)
                nc.vector.tensor_mul(hT[:], sO[:], thc[:])

                # ---------- query = h @ Wq.T + bq ----------
                psum_q = ps_mm.tile([128, 2, B_LOC], f32, space="PSUM", tag="q")
                qT_s = smallp.tile([128, 2, B_LOC], f32, tag="qT")
                for m in range(2):
                    msl = slice(m * 128, (m + 1) * 128)
                    for kc in range(2):
                        nc.tensor.matmul(
                            psum_q[:, m, :],
                            lhsT=wqT_t[:, kc, msl],
                            rhs=hT[:, kc, :],
                            start=(kc == 0),
                            stop=(kc == 1),
                        )
                    nc.vector.tensor_scalar(
                        qT_s[:, m, :], psum_q[:, m, :],
                        bias_q_t[:, m : m + 1], None, op0=Alu.add,
                    )
                # q to natural layout [16, 256]: two PE transposes, one per chunk
                psum_qn = ps_mm.tile([B_LOC, 2 * 128], f32, space="PSUM", tag="qn")
                for m in range(2):
                    nc.tensor.transpose(
                        psum_qn[:, m * 128 : (m + 1) * 128],
                        qT_s[:, m, :],
                        ident_t[:],
                    )
                nc.scalar.copy(qnat[:], psum_qn[:])

                # ---------- attention logits + streaming ----------
                for b in range(B_LOC):
                    # broadcast q[b] to all 128 partitions: one-hot row-select
                    # matmul (lhsT row b all-ones, others zero)
                    psum_qb = ps_tp.tile([128, HD], f32, space="PSUM", tag="qb_ps")
                    nc.tensor.matmul(
                        psum_qb[:],
                        lhsT=bsel_t[:, b, :],
                        rhs=qnat[:],
                        start=True,
                        stop=True,
                    )
                    qb = workp.tile([128, HD], f32, tag="qb")
                    nc.scalar.copy(qb[:], psum_qb[:])

                    if b % 2 == 0:
                        lpair = workp.tile([128, 2 * nchunk], f32, tag="lpair")
                    half = (b % 2) * nchunk

                    for j in range(ndma):
                        et = encp.tile([128, cpd, HD], f32, tag="enc")
                        nc.sync.dma_start(
                            et[:],
                            enc[b, j * cpd * 128 : (j + 1) * cpd * 128, :].rearrange(
                                "(c p) d -> p c d", p=128
                            ),
                        )
                        for c in range(cpd):
                            col = half + j * cpd + c
                            trash = workp.tile([128, HD], f32, tag="trash")
                            # fused multiply + free-axis reduce:
                            # out = (in0 bypass 0) * in1; accum_out = sum(out)
                            nc.vector.scalar_tensor_tensor(
                                out=trash[:],
                                in0=et[:, c, :],
                                scalar=0.0,
                                in1=qb[:],
                                op0=Alu.bypass,
                                op1=Alu.mult,
                                accum_out=lpair[:, col : col + 1],
                            )

                    if b % 2 == 1:
                        # transpose the two-column-block tile: rows become 128
                        # consecutive n values of one b
                        psum_tp = ps_tp.tile([2 * nchunk, 128], f32, space="PSUM", tag="tp")
                        nc.tensor.transpose(psum_tp[:], lpair[:], ident_t[:])
                        ts = workp.tile([2 * nchunk, 128], f32, tag="ts")
                        nc.scalar.copy(ts[:], psum_tp[:])
                        # DRAM logits output (both b of the pair, contiguous)
                        nc.sync.dma_start(
                            logits_out[t, b - 1 : b + 1, :].rearrange(
                                "b (c j) -> (b c) j", j=128
                            ),
                            ts[:],
                        )
                        # SBUF assembly for top-k
                        nc.sync.dma_start(
                            logits16[b - 1 : b, :].rearrange("b (c j) -> b c j", j=128),
                            ts[0:nchunk, :],
                        )
                        nc.sync.dma_start(
                            logits16[b : b + 1, :].rearrange("b (c j) -> b c j", j=128),
                            ts[nchunk : 2 * nchunk, :],
                        )

                # ---------- top-3 ----------
                maxv = smallp.tile([B_LOC, 8], f32, tag="maxv")
                idxv = smallp.tile([B_LOC, 8], u32, tag="idxv")
                nc.vector.max(maxv[:], logits16[:])
                nc.vector.max_index(idxv[:], maxv[:], logits16[:])
                nc.sync.dma_start(idx_out[t, :, :], idxv[:, 0:3])

                # ---------- gather x' = enc[b, top3, :] ----------
                xnat = smallp.tile([B_LOC, 3, HD], f32, tag="xnat")
                for j in range(3):
                    off = smallp.tile([B_LOC, 1], u32, tag=f"off{j}")
                    nc.vector.tensor_add(
                        off[:], idxv[:, j : j + 1], row_base_t[:, 0:1]
                    )
                    nc.gpsimd.indirect_dma_start(
                        out=xnat[:, j, :],
                        out_offset=None,
                        in_=enc[:].rearrange("b n d -> (b n) d"),
                        in_offset=bass.IndirectOffsetOnAxis(ap=off[:, 0:1], axis=0),
                    )
                # transpose gathered rows into xT chunks
                psum_x = ps_mm.tile([128, 6, B_LOC], f32, space="PSUM", tag="xp")
                for j in range(3):
                    for hf in range(2):
                        nc.tensor.transpose(
                            psum_x[:, 2 * j + hf, :],
                            xnat[:, j, hf * 128 : (hf + 1) * 128],
                            ident_t[0:B_LOC, 0:B_LOC],
                        )
                nc.scalar.copy(xT[:], psum_x[:])

    if not nc.is_finalized():
        nc.finalize()
    return nc


def _host_inputs(encoder_outputs, hidden, cell, initial_input, W_ih, b_ih, W_hh,
                 b_hh, Wq, bq, n=N):
    """Per-core input maps (host-side prep: sharding + weight transposes)."""
    f32 = np.float32
    w_ihT = np.ascontiguousarray(W_ih.T.reshape(6, 128, 4 * HD), dtype=f32)
    w_hhT = np.ascontiguousarray(W_hh.T.reshape(2, 128, 4 * HD), dtype=f32)
    wqT = np.ascontiguousarray(Wq.T.reshape(2, 128, HD), dtype=f32)
    bias_g = np.ascontiguousarray((b_ih + b_hh).reshape(8, 128).T, dtype=f32)
    bias_q = np.ascontiguousarray(bq.reshape(2, 128).T, dtype=f32)
    ident = np.eye(128, dtype=f32)
    bsel = np.zeros((B_LOC, B_LOC, 128), dtype=f32)
    for b in range(B_LOC):
        bsel[b, b, :] = 1.0
    row_base = (np.arange(B_LOC, dtype=np.uint64) * n).astype(np.uint32).reshape(
        B_LOC, 1
    )

    in_maps = []
    ncores = encoder_outputs.shape[0] // B_LOC
    for i in range(ncores):
        sl = slice(i * B_LOC, (i + 1) * B_LOC)
        in_maps.append(
            {
                "enc": np.ascontiguousarray(encoder_outputs[sl], dtype=f32),
                "w_ihT": w_ihT,
                "w_hhT": w_hhT,
                "wqT": wqT,
                "bias_g": bias_g,
                "bias_q": bias_q,
                "x0T": np.ascontiguousarray(
                    initial_input[sl].T.reshape(6, 128, B_LOC), dtype=f32
                ),
                "h0T": np.ascontiguousarray(
                    hidden[sl].T.reshape(2, 128, B_LOC), dtype=f32
                ),
                "c0T": np.ascontiguousarray(
                    cell[sl].T.reshape(2, 128, B_LOC), dtype=f32
                ),
                "ident": ident,
                "bsel": bsel,
                "row_base": row_base,
            }
        )
    return in_maps


_NC_CACHE = {}


def kernel(encoder_outputs, hidden, cell, end_node_embed, initial_input,
           W_ih, b_ih, W_hh, b_hh, Wq, bq, max_steps):
    from concourse.bass_utils import run_bass_kernel_spmd

    assert int(max_steps) == S
    encoder_outputs = np.asarray(encoder_outputs)
    in_maps = _host_inputs(
        np.asarray(encoder_outputs), np.asarray(hidden), np.asarray(cell),
        np.asarray(initial_input), np.asarray(W_ih), np.asarray(b_ih),
        np.asarray(W_hh), np.asarray(b_hh), np.asarray(Wq), np.asarray(bq),
    )
    if "nc" not in _NC_CACHE:
        _NC_CACHE["nc"] = _build_nc()
    nc = _NC_CACHE["nc"]
    res = run_bass_kernel_spmd(nc, in_maps, list(range(NCORES)))

    logits_all = np.concatenate(
        [res.results[i]["logits_out"] for i in range(NCORES)], axis=1
    )
    idx_all = np.concatenate(
        [res.results[i]["idx_out"].astype(np.int32) for i in range(NCORES)], axis=1
    )
    return logits_all, idx_all
